# revision 38
# baseline (speedup 1.0000x reference)
"""DKVMN forward kernel for 8 Trainium2 NeuronCores (v9).

Data-parallel over batch: B=128 -> 16 per core, single merged state
v [d=128 partitions, (m,b)=50*16=800 free] bf16. (m,b) column order
(col = m*16 + b) keeps gate broadcasts stride-1 in the last dim so DVE
tensor_tensor runs in 2x 16-bit mode, and makes every m-range a
contiguous column slice so the read-reduction is a pure fold tree of
packed 2x adds (m: 50->25->13->7->4->2->1), no strided reduce.

Per step t:
  w_ps  = sel_t.T @ attnb       PE (bf16) -> PSUM f32  (prefetch 1 step)
  w_sb  = bf16(w_ps)            ACT copy
  z     = v * w_sb              DVE 2x [*,800]
  read  = fold-tree(z)          DVE 2x x6 -> [D,16] bf16
  hps   = w1q@qe (prefetched) + w1r@read   PE
  th    = tanh(hps+b1) bf16     ACT
  a     = tanh(w2ad@th+ab)      PE+ACT (a first: frees t2/q early)
  e     = sigmoid(w2er@th+eb)   PE+ACT
  t2    = w_sb * bc(a)          DVE 2x   (off critical chain)
  q     = v + t2                DVE 2x   (off critical chain)
  t1    = z * bc(e)             DVE 2x
  v'    = q - t1                DVE 2x
"""

import os
import numpy as np
import ml_dtypes
from contextlib import ExitStack

import concourse.bass as bass
import concourse.bacc as bacc
import concourse.mybir as mybir
import concourse.tile as tile
import concourse.bass_utils as bass_utils
from concourse.masks import make_identity

B, S, M, D, NQ = 128, 100, 50, 128, 10000
NCORES = 8
BC = B // NCORES          # 16 batch rows per core
GW = M * BC               # 800 merged state width
NQTILES = (S * BC + 127) // 128   # 13 gather tiles
QCOLS = NQTILES * 128     # 1664

F32 = mybir.dt.float32
BF16 = mybir.dt.bfloat16
I32 = mybir.dt.int32
AF = mybir.ActivationFunctionType
OP = mybir.AluOpType
AX = mybir.AxisListType

_CACHE = {}


def _build_program():
    if "nc" in _CACHE:
        return _CACHE["nc"]

    nc = bacc.Bacc("TRN2", target_bir_lowering=False, debug=False,
                   enable_asserts=False, num_devices=NCORES)

    dram_in = {}
    for name, shape, dt in [
        ("qtb", [D, QCOLS], BF16),
        ("qlast", [D, BC], F32),
        ("kTb", [D, M], BF16),
        ("w1r", [D, D], BF16), ("w1q", [D, D], BF16),
        ("w2er", [D, D], BF16), ("w2ad", [D, D], BF16),
        ("b1", [D, 1], F32), ("eb", [D, 1], F32), ("ab", [D, 1], F32),
        ("ow1r", [D, D], F32), ("ow1q", [D, D], F32),
        ("ob1", [D, 1], F32), ("ow2", [D, 1], F32), ("ob2", [1, 1], F32),
    ]:
        dram_in[name] = nc.dram_tensor(name, shape, dt, kind="ExternalInput").ap()
    pred_out = nc.dram_tensor("pred", [1, BC], F32, kind="ExternalOutput").ap()

    with tile.TileContext(nc) as tc, ExitStack() as ctx:
        persist = ctx.enter_context(tc.tile_pool(name="persist", bufs=1))

        # ---- persistent SBUF tiles ----
        kTb = persist.tile([D, M], BF16, tag="kTb")
        w1r = persist.tile([D, D], BF16, tag="w1r")
        w1q = persist.tile([D, D], BF16, tag="w1q")
        w2er = persist.tile([D, D], BF16, tag="w2er")
        w2ad = persist.tile([D, D], BF16, tag="w2ad")
        b1 = persist.tile([D, 1], F32, tag="b1")
        eb = persist.tile([D, 1], F32, tag="eb")
        ab = persist.tile([D, 1], F32, tag="ab")
        ow1r = persist.tile([D, D], F32, tag="ow1r")
        ow1q = persist.tile([D, D], F32, tag="ow1q")
        ob1 = persist.tile([D, 1], F32, tag="ob1")
        ow2 = persist.tile([D, 1], F32, tag="ow2")
        ob2 = persist.tile([1, 1], F32, tag="ob2")
        ident = persist.tile([128, 128], F32, tag="ident")
        identb = persist.tile([128, 128], BF16, tag="identb")
        qlast = persist.tile([D, BC], F32, tag="qlast")
        qTb = persist.tile([D, QCOLS], BF16, tag="qTb")
        attn = persist.tile([S, GW], F32, tag="attn")
        attnb = persist.tile([S, GW], BF16, tag="attnb")
        vpp = [persist.tile([D, GW], BF16, name=f"vp{p}", tag=f"vp{p}")
               for p in (0, 1)]

        # qtb (the big one) first and alone on the sync queue; spread the
        # small weight loads across the other engines' DGE queues.
        nc.sync.dma_start(qTb[:], dram_in["qtb"][:])
        for i, (nm, t) in enumerate([
                ("kTb", kTb), ("w1r", w1r), ("w1q", w1q),
                ("w2er", w2er), ("w2ad", w2ad), ("b1", b1),
                ("eb", eb), ("ab", ab), ("ow1r", ow1r),
                ("ow1q", ow1q), ("ob1", ob1), ("ow2", ow2),
                ("ob2", ob2), ("qlast", qlast)]):
            eng = (nc.scalar, nc.gpsimd)[i % 2]
            eng.dma_start(t[:], dram_in[nm][:])
        make_identity(nc, ident[:])
        nc.vector.tensor_copy(identb[:], ident[:])
        nc.vector.memset(vpp[0][:], 0.0)

        # ---- phase 2: scores + softmax -> attn[s, (b,m)] f32 ----
        with tc.tile_pool(name="spsum", bufs=4, space="PSUM") as spsum:
            for b in range(BC):
                sc = spsum.tile([S, M], F32, tag="sc")
                qTsl = qTb[:, b:S * BC:BC]        # [128, 100] strided (s,b)
                nc.tensor.matmul(sc[:], qTsl, kTb[:], start=True, stop=True)
                if b % 2 == 0:
                    nc.vector.tensor_copy(attn[:, b * M:(b + 1) * M], sc[:])
                else:
                    nc.scalar.copy(attn[:, b * M:(b + 1) * M], sc[:])

        with tc.tile_pool(name="smx", bufs=1) as smx:
            a3 = attn[:].rearrange("p (b m) -> p b m", b=BC)
            mx = smx.tile([S, BC], F32, tag="mx")
            nc.vector.tensor_reduce(mx[:], a3, axis=AX.X, op=OP.max)
            mxb = mx[:, :, None].broadcast_to([S, BC, M])
            nc.vector.tensor_tensor(a3, a3, mxb, op=OP.subtract)
            nc.scalar.activation(attn[:], attn[:], AF.Exp)
            sm = smx.tile([S, BC], F32, tag="sm")
            nc.vector.tensor_reduce(sm[:], a3, axis=AX.X, op=OP.add)
            rec = smx.tile([S, BC], F32, tag="rec")
            nc.vector.reciprocal(rec[:], sm[:])
            recb = rec[:, :, None].broadcast_to([S, BC, M])
            nc.vector.tensor_tensor(a3, a3, recb, op=OP.mult)
            # reshuffle (b,m) f32 -> (m,b) bf16
            src = attn[:].rearrange("p (b m) -> p b m", b=BC)
            dst = attnb[:].rearrange("p (m b) -> p b m", m=M)
            nc.vector.tensor_copy(dst, src)

        # ---- phase 3: the scan (single merged pipeline) ----
        # fold-tree column sizes for m: 50->25->13->7->4->2->1 (cols x16)
        FOLD_MS = [50, 25, 13, 7, 4, 2, 1]
        with tc.tile_pool(name="wps", bufs=2, space="PSUM") as wps, \
             tc.tile_pool(name="wsb", bufs=3) as wsbp, \
             tc.tile_pool(name="zp", bufs=2) as zp, \
             tc.tile_pool(name="zfp", bufs=2) as zfp, \
             tc.tile_pool(name="wide", bufs=6) as wide, \
             tc.tile_pool(name="small", bufs=12) as small, \
             tc.tile_pool(name="mlp", bufs=2, space="PSUM") as mlpp, \
             tc.tile_pool(name="fin", bufs=1, space="PSUM") as finp:

            wtile = [None] * S
            hqe = [None] * S

            def emit_w(t):
                sel = identb[0:S, t:t + 1].broadcast_to([S, D])
                wpa = wps.tile([D, GW // 2], F32, tag="wpa")
                wpb = wps.tile([D, GW // 2], F32, tag="wpb")
                nc.tensor.matmul(wpa[:], sel, attnb[:, 0:GW // 2],
                                 start=True, stop=True)
                nc.tensor.matmul(wpb[:], sel, attnb[:, GW // 2:GW],
                                 start=True, stop=True)
                w = wsbp.tile([D, GW], BF16, tag="w")
                nc.scalar.copy(w[:, 0:GW // 2], wpa[:])
                nc.scalar.copy(w[:, GW // 2:GW], wpb[:])
                wtile[t] = w

            def emit_qe(t):
                qeT = qTb[:, t * BC:(t + 1) * BC]
                gps = mlpp.tile([D, 3 * BC], F32, tag="hps")
                nc.tensor.matmul(gps[:, 0:BC], w1q[:], qeT, start=True,
                                 stop=False)
                hqe[t] = gps

            def fold_tree(z, pool, tag):
                """Packed 2x adds folding m 50 -> 1; returns [D, BC] bf16."""
                cur, mcur = z, FOLD_MS[0]
                outs = []
                for mn in FOLD_MS[1:]:
                    lo = mcur - mn            # columns folded onto the tail
                    o = pool.tile([D, mn * BC], BF16, tag=f"{tag}{mn}")
                    nc.vector.tensor_tensor(
                        o[:, (mn - lo) * BC:], cur[:, (mn - lo) * BC:mn * BC],
                        cur[:, mn * BC:mcur * BC], op=OP.add)
                    if mn > lo:
                        nc.vector.tensor_copy(o[:, 0:(mn - lo) * BC],
                                              cur[:, 0:(mn - lo) * BC])
                    cur, mcur = o, mn
                return cur

            def emit_step(t):
                w = wtile[t]
                vcur, vnext = vpp[t % 2], vpp[(t + 1) % 2]
                z = zp.tile([D, GW], BF16, tag="z")
                nc.vector.tensor_tensor(z[:], vcur[:], w[:], op=OP.mult)
                read = fold_tree(z, zfp, "f")
                gps = hqe[t]
                hqe[t] = None
                nc.tensor.matmul(gps[:, 0:BC], w1r[:], read[:], start=False,
                                 stop=True)
                th = small.tile([D, BC], BF16, tag="th")
                nc.scalar.activation(th[:], gps[:, 0:BC], AF.Tanh, bias=b1[:])
                # a first: t2/q come off the critical chain early
                nc.tensor.matmul(gps[:, 2 * BC:3 * BC], w2ad[:], th[:],
                                 start=True, stop=True)
                a = small.tile([D, BC], BF16, tag="a")
                nc.scalar.activation(a[:], gps[:, 2 * BC:3 * BC], AF.Tanh,
                                     bias=ab[:])
                nc.tensor.matmul(gps[:, BC:2 * BC], w2er[:], th[:],
                                 start=True, stop=True)
                e = small.tile([D, BC], BF16, tag="e")
                nc.scalar.activation(e[:], gps[:, BC:2 * BC], AF.Sigmoid,
                                     bias=eb[:])
                abc = a[:, None, :].broadcast_to([D, M, BC])
                t2 = wide.tile([D, GW], BF16, tag="t2")
                nc.vector.tensor_tensor(t2[:].rearrange("p (m b) -> p m b", m=M),
                                        w[:].rearrange("p (m b) -> p m b", m=M),
                                        abc, op=OP.mult)
                q = wide.tile([D, GW], BF16, tag="q")
                nc.vector.tensor_tensor(q[:], vcur[:], t2[:], op=OP.add)
                ebc = e[:, None, :].broadcast_to([D, M, BC])
                t1 = wide.tile([D, GW], BF16, tag="t1")
                nc.vector.tensor_tensor(t1[:].rearrange("p (m b) -> p m b", m=M),
                                        z[:].rearrange("p (m b) -> p m b", m=M),
                                        ebc, op=OP.mult)
                nc.vector.tensor_tensor(vnext[:], q[:], t1[:], op=OP.subtract)
                wtile[t] = None

            emit_w(0)
            emit_qe(0)
            for t in range(S):
                emit_step(t)
                if t + 1 < S:
                    emit_w(t + 1)
                    emit_qe(t + 1)

            # ---- final prediction (w from t=S-1, v after last update) ----
            sel = identb[0:S, S - 1:S].broadcast_to([S, D])
            wpa = wps.tile([D, GW // 2], F32, tag="wpa")
            wpb = wps.tile([D, GW // 2], F32, tag="wpb")
            nc.tensor.matmul(wpa[:], sel, attnb[:, 0:GW // 2],
                             start=True, stop=True)
            nc.tensor.matmul(wpb[:], sel, attnb[:, GW // 2:GW],
                             start=True, stop=True)
            wf = wsbp.tile([D, GW], BF16, tag="w")
            nc.scalar.copy(wf[:, 0:GW // 2], wpa[:])
            nc.scalar.copy(wf[:, GW // 2:GW], wpb[:])
            zfin = zp.tile([D, GW], BF16, tag="z")
            nc.vector.tensor_tensor(zfin[:], vpp[S % 2][:], wf[:], op=OP.mult)
            readFb = fold_tree(zfin, zfp, "g")
            readF = small.tile([D, BC], F32, tag="readF")
            nc.scalar.copy(readF[:], readFb[:])
            h2ps = finp.tile([D, BC], F32, tag="fin")
            nc.tensor.matmul(h2ps[:], ow1r[:], readF[:], start=True, stop=False)
            nc.tensor.matmul(h2ps[:], ow1q[:], qlast[:], start=False, stop=True)
            h2 = small.tile([D, BC], F32, tag="h2")
            nc.scalar.activation(h2[:], h2ps[:], AF.Relu, bias=ob1[:])
            pps = mlpp.tile([D, 3 * BC], F32, tag="hps")
            nc.tensor.matmul(pps[0:1, 0:BC], ow2[:], h2[:], start=True,
                             stop=True)
            ps = small.tile([1, BC], F32, tag="pred")
            nc.scalar.activation(ps[:], pps[0:1, 0:BC], AF.Sigmoid, bias=ob2[:])
            nc.sync.dma_start(pred_out[:], ps[:])

    nc.compile()
    _CACHE["nc"] = nc
    return nc


def _host_inputs(inputs):
    """Per-core input maps from the full problem inputs."""
    q = np.asarray(inputs["question_seq"]).astype(np.int64)
    emb = np.ascontiguousarray(np.asarray(inputs["emb"], dtype=np.float32))
    key_matrix = np.asarray(inputs["key_matrix"], dtype=np.float32)
    vu_w1 = np.asarray(inputs["vu_w1"], dtype=np.float32)
    vu_b1 = np.asarray(inputs["vu_b1"], dtype=np.float32)
    vu_w2 = np.asarray(inputs["vu_w2"], dtype=np.float32)
    vu_b2 = np.asarray(inputs["vu_b2"], dtype=np.float32)
    er_w = np.asarray(inputs["er_w"], dtype=np.float32)
    er_b = np.asarray(inputs["er_b"], dtype=np.float32)
    ad_w = np.asarray(inputs["ad_w"], dtype=np.float32)
    ad_b = np.asarray(inputs["ad_b"], dtype=np.float32)
    out_w1 = np.asarray(inputs["out_w1"], dtype=np.float32)
    out_b1 = np.asarray(inputs["out_b1"], dtype=np.float32)
    out_w2 = np.asarray(inputs["out_w2"], dtype=np.float32)
    out_b2 = np.asarray(inputs["out_b2"], dtype=np.float32)

    w2er = (vu_w2.astype(np.float64) @ er_w.astype(np.float64)).astype(np.float32)
    w2ad = (vu_w2.astype(np.float64) @ ad_w.astype(np.float64)).astype(np.float32)
    ebf = (vu_b2.astype(np.float64) @ er_w.astype(np.float64) + er_b).astype(np.float32)
    abf = (vu_b2.astype(np.float64) @ ad_w.astype(np.float64) + ad_b).astype(np.float32)

    bf = ml_dtypes.bfloat16
    shared = {
        "kTb": np.ascontiguousarray(key_matrix.T).astype(bf),
        "w1r": np.ascontiguousarray(vu_w1[:D]).astype(bf),
        "w1q": np.ascontiguousarray(vu_w1[D:]).astype(bf),
        "w2er": w2er.astype(bf), "w2ad": w2ad.astype(bf),
        "b1": vu_b1.reshape(D, 1), "eb": ebf.reshape(D, 1), "ab": abf.reshape(D, 1),
        "ow1r": np.ascontiguousarray(out_w1[:D]),
        "ow1q": np.ascontiguousarray(out_w1[D:]),
        "ob1": out_b1.reshape(D, 1),
        "ow2": np.ascontiguousarray(out_w2.reshape(D, 1)),
        "ob2": out_b2.reshape(1, 1),
    }
    in_maps = []
    for c in range(NCORES):
        qc = q[c * BC:(c + 1) * BC, :]          # [BC, S]
        idxs = qc.T.reshape(-1)                  # n = s*BC + b order
        qg = emb[idxs]                           # [S*BC, D]
        qtb = np.zeros((D, QCOLS), np.float32)
        qtb[:, :S * BC] = qg.T
        m = dict(shared)
        m["qtb"] = qtb.astype(bf)
        m["qlast"] = np.ascontiguousarray(qg[(S - 1) * BC:, :].T)
        in_maps.append(m)
    return in_maps


def _install_ntff_shim():
    # Optional: enables NTFF hardware profiling under axon when tracing is
    # requested. Harmless no-op if the pieces are missing.
    import types, sys
    if "antenv.axon_hooks" in sys.modules:
        return
    try:
        import antenv
        from trn_agent_boot.trn_boot import _ntff_profile_via_ctypes
        hook = _ntff_profile_via_ctypes("/opt/axon/libaxon_pjrt.so")
        mod = types.ModuleType("antenv.axon_hooks")
        state = {"hook": hook}
        mod.get_axon_ntff_profile_hook = lambda: state["hook"]
        mod.set_axon_ntff_profile_hook = lambda h: state.update(hook=h)
        sys.modules["antenv.axon_hooks"] = mod
        antenv.axon_hooks = mod
    except Exception:
        pass


def kernel(**inputs) -> np.ndarray:
    if bool(int(os.environ.get("DKVMN_TRACE", "0"))):
        _install_ntff_shim()
    nc = _build_program()
    in_maps = _host_inputs(inputs)
    res = bass_utils.run_bass_kernel_spmd(
        nc, in_maps, core_ids=list(range(NCORES)),
        trace=bool(int(os.environ.get("DKVMN_TRACE", "0"))),
    )
    _CACHE["last_results"] = res
    pred = np.concatenate([res.results[c]["pred"].reshape(BC) for c in range(NCORES)])
    return pred.astype(np.float32)
